# revision 5
# baseline (speedup 1.0000x reference)
"""Trainium2 Bass kernel for the NRI encoder problem.

Math: the reference's construct_pair makes pair[n,i,j,:] = concat(h[n,i], h[n,i])
(independent of the receiver axis j), so the (m,m) edge grid collapses:
  edge[n,i,j,:]   = E[n,i,:]            where E = f_edge(concat(h,h))
  e2n[n,j,:]      = sum_{i!=j} E[n,i,:] = S[n] - E[n,j,:],  S[n] = sum_i E[n,i]
  h2              = f_e2n(e2n)
  edge2[n,i,j]    = f_n2e(concat(h2[n,i],h2[n,i])) / m      (broadcast over j)

Sharding: 8 cores, each handles 128 of the 1024 (batch, node) rows. Every core
redundantly computes h/E/S for its whole batch element (tiny: m=512, L=128),
then runs the last two FFNs and the output writes only for its own 128 nodes.
The per-core input x slice is rolled so the core's nodes sit at positions
0..127; the global sum S is permutation invariant, so results are exact.

Layout is feature-major on chip (features on partitions, nodes on the free
axis) so no activation ever needs transposing; weights (din,dout) are already
in matmul lhsT layout. concat(h,h) @ W is folded host-side to h @ (W_top+W_bot).
"""

import numpy as np

L = 128
M = 512
N_B = 2
D_IN = 6
N_CORES = 8
RPC = 128  # rows (nodes) per core

# pack column offsets (all 128-partition chunks)
_GA = 0            # 10 bias cols + Wn2 (2x128)
_GB = 266          # We1f (128) + We2 (128)
_GC = 522          # Wen1 (2x128) + Wen2 (2x128)
_GD = 1034         # Wne1f (128) + Wne2 (1)
_NPACK = 1163

_CACHE = {}


def _ensure_path():
    try:
        import concourse  # noqa: F401
    except ImportError:
        import sys
        for p in ("/opt/trn_rl_repo", "/root/.axon_site/_ro/trn_rl_repo"):
            if p not in sys.path:
                sys.path.insert(0, p)


def _build_bass():
    _ensure_path()
    import concourse.mybir as mybir
    from concourse import bacc
    from concourse.tile import TileContext
    from concourse.masks import make_identity

    f32 = mybir.dt.float32
    AF = mybir.ActivationFunctionType
    OP = mybir.AluOpType

    nc = bacc.Bacc()
    pack_d = nc.declare_dram_parameter("pack", [128, _NPACK], f32, isOutput=False)
    small_d = nc.declare_dram_parameter("small", [D_IN, 256 + M], f32, isOutput=False)
    h2o_d = nc.declare_dram_parameter("h2o", [RPC, L], f32, isOutput=True)
    edo_d = nc.declare_dram_parameter("edo", [RPC, M], f32, isOutput=True)

    with TileContext(nc) as tc:
        with (
            tc.tile_pool(name="w", bufs=1) as wp,
            tc.tile_pool(name="act", bufs=1) as sp,
            tc.tile_pool(name="psA", bufs=3, space="PSUM") as ppa,
            tc.tile_pool(name="psB", bufs=3, space="PSUM") as ppb,
        ):
            small = wp.tile([D_IN, 256 + M], f32)
            ga = wp.tile([128, 266], f32)
            gb = wp.tile([128, 256], f32)
            gc = wp.tile([128, 512], f32)
            gd = wp.tile([128, 129], f32)
            ident = wp.tile([128, 128], f32)

            nc.sync.dma_start(out=small[:], in_=small_d[:])
            nc.sync.dma_start(out=ga[:], in_=pack_d[:, _GA:_GA + 266])
            nc.sync.dma_start(out=gb[:], in_=pack_d[:, _GB:_GB + 256])
            nc.sync.dma_start(out=gc[:], in_=pack_d[:, _GC:_GC + 512])
            nc.sync.dma_start(out=gd[:], in_=pack_d[:, _GD:_GD + 129])
            make_identity(nc, ident[:])

            xT = small[0:D_IN, 256:256 + M]
            b = lambda i: ga[:, i:i + 1]  # noqa: E731  bias column i

            # ACT instructions encode at most one sync wait; have ACT observe
            # the ga DMA here so later activations only wait on PE.
            warm = sp.tile([128, 1], f32)
            nc.scalar.copy(warm[:], ga[:, 0:1])

            # f_node layer 1: H1t = relu(Wn1^T xT + bn1), split over 2L
            ps_h1a = ppa.tile([128, M], f32, tag="psA")
            nc.tensor.matmul(ps_h1a[:], lhsT=small[0:D_IN, 0:128], rhs=xT)
            h1a = sp.tile([128, M], f32)
            nc.scalar.activation(h1a[:], ps_h1a[:], AF.Relu, bias=b(0), scale=1.0)

            ps_h1b = ppa.tile([128, M], f32, tag="psA")
            nc.tensor.matmul(ps_h1b[:], lhsT=small[0:D_IN, 128:256], rhs=xT)
            h1b = sp.tile([128, M], f32)
            nc.scalar.activation(h1b[:], ps_h1b[:], AF.Relu, bias=b(1), scale=1.0)

            # f_node layer 2: Ht = Wn2^T H1 + bn2 (accumulate the 2L halves)
            ps_h = ppa.tile([128, M], f32, tag="psA")
            nc.tensor.matmul(ps_h[:], lhsT=ga[:, 10:138], rhs=h1a[:], start=True, stop=False)
            nc.tensor.matmul(ps_h[:], lhsT=ga[:, 138:266], rhs=h1b[:], start=False, stop=True)
            ht = sp.tile([128, M], f32)
            nc.vector.tensor_scalar_add(ht[:], ps_h[:], b(2))

            # f_edge layer 1: A1t = relu(We1f^T Ht + be1)
            ps_a1 = ppa.tile([128, M], f32, tag="psA")
            nc.tensor.matmul(ps_a1[:], lhsT=gb[:, 0:128], rhs=ht[:])
            a1 = sp.tile([128, M], f32)
            nc.scalar.activation(a1[:], ps_a1[:], AF.Relu, bias=b(3), scale=1.0)

            # f_edge layer 2: Et = We2^T A1 + be2; ssum = row-sum (fused accum)
            ps_e = ppa.tile([128, M], f32, tag="psA")
            nc.tensor.matmul(ps_e[:], lhsT=gb[:, 128:256], rhs=a1[:])
            et = sp.tile([128, M], f32)
            ssum = sp.tile([128, 1], f32)
            nc.vector.tensor_scalar(
                out=et[:], in0=ps_e[:], scalar1=b(4), scalar2=None,
                op0=OP.add, op1=OP.add, accum_out=ssum[:],
            )

            # e2n = S - E, own nodes only (first RPC columns)
            e2n = sp.tile([128, RPC], f32)
            nc.vector.tensor_sub(
                e2n[:], ssum[:].broadcast_to([128, RPC]), et[:, 0:RPC]
            )

            # f_e2n layer 1: A2 = relu(Wen1^T e2n + ben1), split over 2L
            ps_a2a = ppb.tile([128, RPC], f32, tag="psB")
            nc.tensor.matmul(ps_a2a[:], lhsT=gc[:, 0:128], rhs=e2n[:])
            a2a = sp.tile([128, RPC], f32)
            nc.scalar.activation(a2a[:], ps_a2a[:], AF.Relu, bias=b(5), scale=1.0)

            ps_a2b = ppb.tile([128, RPC], f32, tag="psB")
            nc.tensor.matmul(ps_a2b[:], lhsT=gc[:, 128:256], rhs=e2n[:])
            a2b = sp.tile([128, RPC], f32)
            nc.scalar.activation(a2b[:], ps_a2b[:], AF.Relu, bias=b(6), scale=1.0)

            # f_e2n layer 2: H2t = Wen2^T A2 + ben2
            ps_h2 = ppb.tile([128, RPC], f32, tag="psB")
            nc.tensor.matmul(ps_h2[:], lhsT=gc[:, 256:384], rhs=a2a[:], start=True, stop=False)
            nc.tensor.matmul(ps_h2[:], lhsT=gc[:, 384:512], rhs=a2b[:], start=False, stop=True)
            h2t = sp.tile([128, RPC], f32)
            nc.vector.tensor_scalar_add(h2t[:], ps_h2[:], b(7))

            # f_node2edge layer 1: A3 = relu(Wne1f^T H2t + bne1)
            ps_a3 = ppb.tile([128, RPC], f32, tag="psB")
            nc.tensor.matmul(ps_a3[:], lhsT=gd[:, 0:128], rhs=h2t[:])
            a3 = sp.tile([128, RPC], f32)
            nc.scalar.activation(a3[:], ps_a3[:], AF.Relu, bias=b(8), scale=1.0)

            # f_node2edge layer 2 (per own node): s = (A3^T Wne2)/m + bne2/m
            ps_s = ppb.tile([RPC, 1], f32, tag="psB")
            nc.tensor.matmul(ps_s[:], lhsT=a3[:], rhs=gd[:, 128:129])
            scol = sp.tile([RPC, 1], f32)
            nc.vector.tensor_scalar(
                out=scol[:], in0=ps_s[:], scalar1=1.0 / M, scalar2=b(9),
                op0=OP.mult, op1=OP.add,
            )

            # edge2 rows: each own node's scalar broadcast across all m columns
            edt = sp.tile([RPC, M], f32)
            nc.vector.tensor_copy(out=edt[:], in_=scol[:].broadcast_to([RPC, M]))
            nc.sync.dma_start(out=edo_d[:], in_=edt[:])

            # h2 slice: transpose (features, nodes) -> (nodes, features)
            ps_h2n = ppb.tile([RPC, L], f32, tag="psB")
            nc.tensor.transpose(ps_h2n[:], h2t[:], ident[:])
            h2n = sp.tile([RPC, L], f32)
            nc.scalar.copy(h2n[:], ps_h2n[:])
            nc.sync.dma_start(out=h2o_d[:], in_=h2n[:])

    if not nc.is_finalized():
        nc.finalize()
    return nc


def _prep_pack(Wn2, We1, We2, Wen1, Wen2, Wne1, Wne2,
               bn1, bn2, be1, be2, ben1, ben2, bne1, bne2):
    pack = np.zeros((128, _NPACK), np.float32)
    pack[:, 0] = bn1[:128]
    pack[:, 1] = bn1[128:]
    pack[:, 2] = bn2
    pack[:, 3] = be1
    pack[:, 4] = be2
    pack[:, 5] = ben1[:128]
    pack[:, 6] = ben1[128:]
    pack[:, 7] = ben2
    pack[:, 8] = bne1
    pack[:, 9] = bne2[0] / np.float32(M)
    pack[:, 10:138] = Wn2[:128]
    pack[:, 138:266] = Wn2[128:]
    pack[:, _GB:_GB + 128] = We1[:128] + We1[128:]
    pack[:, _GB + 128:_GB + 256] = We2
    pack[:, _GC:_GC + 128] = Wen1[:, :128]
    pack[:, _GC + 128:_GC + 256] = Wen1[:, 128:]
    pack[:, _GC + 256:_GC + 384] = Wen2[:128]
    pack[:, _GC + 384:_GC + 512] = Wen2[128:]
    pack[:, _GD:_GD + 128] = Wne1[:128] + Wne1[128:]
    pack[:, _GD + 128] = Wne2[:, 0]
    return pack


def kernel(x, Wn1, bn1, Wn2, bn2, We1, be1, We2, be2,
           Wen1, ben1, Wen2, ben2, Wne1, bne1, Wne2, bne2):
    _ensure_path()
    from concourse.bass_utils import run_bass_kernel_spmd

    f = np.float32
    x = np.asarray(x, f)
    args = [np.asarray(a, f) for a in (
        Wn2, We1, We2, Wen1, Wen2, Wne1, Wne2,
        bn1, bn2, be1, be2, ben1, ben2, bne1, bne2)]
    Wn1 = np.asarray(Wn1, f)
    pack = _prep_pack(*args)

    in_maps = []
    for k in range(N_CORES):
        n, r = divmod(k, N_CORES // N_B)
        small = np.empty((D_IN, 256 + M), f)
        small[:, 0:256] = Wn1
        small[:, 256:] = np.roll(x[n], -r * RPC, axis=0).T
        in_maps.append({"pack": pack, "small": small})

    if "nc" not in _CACHE:
        _CACHE["nc"] = _build_bass()
    res = run_bass_kernel_spmd(_CACHE["nc"], in_maps, list(range(N_CORES))).results

    h2 = np.empty((N_B, M, L), f)
    edge2 = np.empty((N_B, M, M), f)
    for k in range(N_CORES):
        n, r = divmod(k, N_CORES // N_B)
        h2[n, r * RPC:(r + 1) * RPC] = res[k]["h2o"]
        edge2[n, r * RPC:(r + 1) * RPC] = res[k]["edo"]
    return h2, edge2


# revision 10
# speedup vs baseline: 1.1790x; 1.1790x over previous
"""Trainium2 Bass kernel for the NRI encoder problem.

Math: the reference's construct_pair makes pair[n,i,j,:] = concat(h[n,i], h[n,i])
(independent of the receiver axis j), so the (m,m) edge grid collapses:
  edge[n,i,j,:]   = E[n,i,:]            where E = f_edge(concat(h,h))
  e2n[n,j,:]      = sum_{i!=j} E[n,i,:] = S[n] - E[n,j,:],  S[n] = sum_i E[n,i]
  h2              = f_e2n(e2n)
  edge2[n,i,j]    = f_n2e(concat(h2[n,i],h2[n,i])) / m      (broadcast over j)

Sharding: 8 cores, each handles 128 of the 1024 (batch, node) rows. Every core
redundantly computes h/E/S for its whole batch element (tiny: m=512, L=128),
then runs the last two FFNs and the output writes only for its own 128 nodes.
The per-core input x slice is rolled so the core's nodes sit at positions
0..127; the global sum S is permutation invariant, so results are exact.

Layout is feature-major on chip (features on partitions, nodes on the free
axis) so no activation ever needs transposing; weights (din,dout) are already
in matmul lhsT layout. concat(h,h) @ W is folded host-side to h @ (W_top+W_bot).

Perf notes (from perfetto traces): matmuls run as float32r (single PE pass vs
fp32's two half-rate passes); input DMAs are issued from three different
engines' queues so they land before the compute chain needs them; the two
output DMAs are split across queues to shorten the tail.
"""

import numpy as np

L = 128
M = 512
N_B = 2
D_IN = 6
N_CORES = 8
RPC = 128  # rows (nodes) per core

# pack column offsets (all 128-partition chunks)
_W2 = 0            # Wn2 (2x128)
_GB = 256          # We1f (128) + We2 (128)
_GC = 512          # Wen1 (2x128) + Wen2 (2x128)
_GD = 1024         # Wne1f (128) + Wne2 (1)
_NPACK = 1153

_CACHE = {}


def _ensure_path():
    try:
        import concourse  # noqa: F401
    except ImportError:
        import sys
        for p in ("/opt/trn_rl_repo", "/root/.axon_site/_ro/trn_rl_repo"):
            if p not in sys.path:
                sys.path.insert(0, p)


def _build_bass():
    _ensure_path()
    import concourse.mybir as mybir
    from concourse import bacc
    from concourse.tile import TileContext
    from concourse.masks import make_identity

    f32 = mybir.dt.float32
    f32r = mybir.dt.float32r
    AF = mybir.ActivationFunctionType
    OP = mybir.AluOpType

    nc = bacc.Bacc()
    pack_d = nc.declare_dram_parameter("pack", [128, _NPACK], f32r, isOutput=False)
    bias_d = nc.declare_dram_parameter("bias", [128, 10], f32, isOutput=False)
    small_d = nc.declare_dram_parameter("small", [D_IN, 256 + M], f32r, isOutput=False)
    h2o_d = nc.declare_dram_parameter("h2o", [RPC, L], f32, isOutput=True)
    edo_d = nc.declare_dram_parameter("edo", [RPC, M], f32, isOutput=True)

    with TileContext(nc) as tc:
        with (
            tc.tile_pool(name="w", bufs=1) as wp,
            tc.tile_pool(name="act", bufs=1) as sp,
            tc.tile_pool(name="psA", bufs=3, space="PSUM") as ppa,
            tc.tile_pool(name="psB", bufs=3, space="PSUM") as ppb,
        ):
            small = wp.tile([D_IN, 256 + M], f32r)
            bias = wp.tile([128, 10], f32)
            w2 = wp.tile([128, 256], f32r)
            gb = wp.tile([128, 256], f32r)
            gc = wp.tile([128, 512], f32r)
            gd = wp.tile([128, 129], f32r)
            ident = wp.tile([128, 128], f32)

            # spread input DMAs over three queues, critical-path first
            nc.sync.dma_start(out=small[:], in_=small_d[:])
            nc.sync.dma_start(out=bias[:], in_=bias_d[:])
            nc.sync.dma_start(out=w2[:], in_=pack_d[:, _W2:_W2 + 256])
            nc.scalar.dma_start(out=gb[:], in_=pack_d[:, _GB:_GB + 256])
            nc.scalar.dma_start(out=gc[:], in_=pack_d[:, _GC:_GC + 512])
            nc.scalar.dma_start(out=gd[:], in_=pack_d[:, _GD:_GD + 129])
            make_identity(nc, ident[:])

            xT = small[0:D_IN, 256:256 + M]
            b = lambda i: bias[:, i:i + 1]  # noqa: E731  bias column i

            # ACT instructions encode one sync wait; observe the bias DMA here
            # so later activations only wait on PE.
            warm = sp.tile([128, 1], f32)
            nc.scalar.copy(warm[:], bias[:, 0:1])

            # f_node layer 1: H1t = relu(Wn1^T xT + bn1), halves on ACT / DVE
            ps_h1a = ppa.tile([128, M], f32, tag="psA")
            nc.tensor.matmul(ps_h1a[:], lhsT=small[0:D_IN, 0:128], rhs=xT)
            h1a = sp.tile([128, M], f32r)
            nc.scalar.activation(h1a[:], ps_h1a[:], AF.Relu, bias=b(0), scale=1.0)

            ps_h1b = ppa.tile([128, M], f32, tag="psA")
            nc.tensor.matmul(ps_h1b[:], lhsT=small[0:D_IN, 128:256], rhs=xT)
            h1b = sp.tile([128, M], f32r)
            nc.vector.tensor_scalar(
                out=h1b[:], in0=ps_h1b[:], scalar1=b(1), scalar2=0.0,
                op0=OP.add, op1=OP.max,
            )

            # f_node layer 2: Ht = Wn2^T H1 + bn2 (accumulate the 2L halves)
            ps_h = ppa.tile([128, M], f32, tag="psA")
            nc.tensor.matmul(ps_h[:], lhsT=w2[:, 0:128], rhs=h1a[:],
                             start=True, stop=False)
            nc.tensor.matmul(ps_h[:], lhsT=w2[:, 128:256], rhs=h1b[:],
                             start=False, stop=True)
            ht = sp.tile([128, M], f32r)
            nc.scalar.activation(ht[:], ps_h[:], AF.Identity, bias=b(2), scale=1.0)

            # f_edge layer 1: A1t = relu(We1f^T Ht + be1)
            ps_a1 = ppa.tile([128, M], f32, tag="psA")
            nc.tensor.matmul(ps_a1[:], lhsT=gb[:, 0:128], rhs=ht[:])
            a1 = sp.tile([128, M], f32r)
            nc.scalar.activation(a1[:], ps_a1[:], AF.Relu, bias=b(3), scale=1.0)

            # f_edge layer 2: Et = We2^T A1 + be2; ssum = row-sum (fused accum)
            ps_e = ppa.tile([128, M], f32, tag="psA")
            nc.tensor.matmul(ps_e[:], lhsT=gb[:, 128:256], rhs=a1[:])
            et = sp.tile([128, M], f32)
            ssum = sp.tile([128, 1], f32)
            nc.vector.tensor_scalar(
                out=et[:], in0=ps_e[:], scalar1=b(4), scalar2=None,
                op0=OP.add, op1=OP.add, accum_out=ssum[:],
            )

            # e2n = S - E, own nodes only (first RPC columns)
            e2n = sp.tile([128, RPC], f32r)
            nc.vector.tensor_sub(
                e2n[:], ssum[:].broadcast_to([128, RPC]), et[:, 0:RPC]
            )

            # f_e2n layer 1: A2 = relu(Wen1^T e2n + ben1), halves on ACT / DVE
            ps_a2a = ppb.tile([128, RPC], f32, tag="psB")
            nc.tensor.matmul(ps_a2a[:], lhsT=gc[:, 0:128], rhs=e2n[:])
            a2a = sp.tile([128, RPC], f32r)
            nc.scalar.activation(a2a[:], ps_a2a[:], AF.Relu, bias=b(5), scale=1.0)

            ps_a2b = ppb.tile([128, RPC], f32, tag="psB")
            nc.tensor.matmul(ps_a2b[:], lhsT=gc[:, 128:256], rhs=e2n[:])
            a2b = sp.tile([128, RPC], f32r)
            nc.vector.tensor_scalar(
                out=a2b[:], in0=ps_a2b[:], scalar1=b(6), scalar2=0.0,
                op0=OP.add, op1=OP.max,
            )

            # f_e2n layer 2: H2t = Wen2^T A2 + ben2
            ps_h2 = ppb.tile([128, RPC], f32, tag="psB")
            nc.tensor.matmul(ps_h2[:], lhsT=gc[:, 256:384], rhs=a2a[:],
                             start=True, stop=False)
            nc.tensor.matmul(ps_h2[:], lhsT=gc[:, 384:512], rhs=a2b[:],
                             start=False, stop=True)
            h2t = sp.tile([128, RPC], f32r)
            nc.scalar.activation(h2t[:], ps_h2[:], AF.Identity, bias=b(7), scale=1.0)

            # h2 slice out: transpose (features, nodes) -> (nodes, features)
            ps_h2n = ppb.tile([RPC, L], f32, tag="psB")
            nc.tensor.transpose(ps_h2n[:], h2t[:].bitcast(f32), ident[:])
            h2n = sp.tile([RPC, L], f32)
            nc.scalar.copy(h2n[:], ps_h2n[:])
            nc.sync.dma_start(out=h2o_d[:], in_=h2n[:])

            # f_node2edge layer 1: A3 = relu(Wne1f^T H2t + bne1)
            ps_a3 = ppb.tile([128, RPC], f32, tag="psB")
            nc.tensor.matmul(ps_a3[:], lhsT=gd[:, 0:128], rhs=h2t[:])
            a3 = sp.tile([128, RPC], f32r)
            nc.scalar.activation(a3[:], ps_a3[:], AF.Relu, bias=b(8), scale=1.0)

            # f_node2edge layer 2 (per own node): s = (A3^T Wne2)/m + bne2/m
            ps_s = ppb.tile([RPC, 1], f32, tag="psB")
            nc.tensor.matmul(ps_s[:], lhsT=a3[:].bitcast(f32),
                             rhs=gd[:, 128:129].bitcast(f32))
            scol = sp.tile([RPC, 1], f32)
            nc.vector.tensor_scalar(
                out=scol[:], in0=ps_s[:], scalar1=1.0 / M, scalar2=b(9),
                op0=OP.mult, op1=OP.add,
            )

            # edge2 rows: own node's scalar broadcast across all m columns;
            # write the two halves on separate queues to shorten the tail
            edt = sp.tile([RPC, M], f32)
            nc.vector.tensor_copy(out=edt[:], in_=scol[:].broadcast_to([RPC, M]))
            nc.sync.dma_start(out=edo_d[:, 0:256], in_=edt[:, 0:256])
            nc.scalar.dma_start(out=edo_d[:, 256:512], in_=edt[:, 256:512])

    if not nc.is_finalized():
        nc.finalize()
    return nc


def _prep_pack(Wn2, We1, We2, Wen1, Wen2, Wne1, Wne2):
    pack = np.zeros((128, _NPACK), np.float32)
    pack[:, 0:128] = Wn2[:128]
    pack[:, 128:256] = Wn2[128:]
    pack[:, _GB:_GB + 128] = We1[:128] + We1[128:]
    pack[:, _GB + 128:_GB + 256] = We2
    pack[:, _GC:_GC + 128] = Wen1[:, :128]
    pack[:, _GC + 128:_GC + 256] = Wen1[:, 128:]
    pack[:, _GC + 256:_GC + 384] = Wen2[:128]
    pack[:, _GC + 384:_GC + 512] = Wen2[128:]
    pack[:, _GD:_GD + 128] = Wne1[:128] + Wne1[128:]
    pack[:, _GD + 128] = Wne2[:, 0]
    return pack


def _prep_bias(bn1, bn2, be1, be2, ben1, ben2, bne1, bne2):
    bias = np.zeros((128, 10), np.float32)
    bias[:, 0] = bn1[:128]
    bias[:, 1] = bn1[128:]
    bias[:, 2] = bn2
    bias[:, 3] = be1
    bias[:, 4] = be2
    bias[:, 5] = ben1[:128]
    bias[:, 6] = ben1[128:]
    bias[:, 7] = ben2
    bias[:, 8] = bne1
    bias[:, 9] = bne2[0] / np.float32(M)
    return bias


def kernel(x, Wn1, bn1, Wn2, bn2, We1, be1, We2, be2,
           Wen1, ben1, Wen2, ben2, Wne1, bne1, Wne2, bne2):
    _ensure_path()
    from concourse.bass_utils import run_bass_kernel_spmd

    f = np.float32
    x = np.asarray(x, f)
    pack = _prep_pack(*[np.asarray(a, f) for a in
                        (Wn2, We1, We2, Wen1, Wen2, Wne1, Wne2)])
    bias = _prep_bias(*[np.asarray(a, f) for a in
                        (bn1, bn2, be1, be2, ben1, ben2, bne1, bne2)])
    Wn1 = np.asarray(Wn1, f)

    in_maps = []
    for k in range(N_CORES):
        n, r = divmod(k, N_CORES // N_B)
        small = np.empty((D_IN, 256 + M), f)
        small[:, 0:256] = Wn1
        small[:, 256:] = np.roll(x[n], -r * RPC, axis=0).T
        in_maps.append({"pack": pack, "bias": bias, "small": small})

    if "nc" not in _CACHE:
        _CACHE["nc"] = _build_bass()
    res = run_bass_kernel_spmd(_CACHE["nc"], in_maps, list(range(N_CORES))).results

    h2 = np.empty((N_B, M, L), f)
    edge2 = np.empty((N_B, M, M), f)
    for k in range(N_CORES):
        n, r = divmod(k, N_CORES // N_B)
        h2[n, r * RPC:(r + 1) * RPC] = res[k]["h2o"]
        edge2[n, r * RPC:(r + 1) * RPC] = res[k]["edo"]
    return h2, edge2
